# revision 75
# baseline (speedup 1.0000x reference)
"""Trainium2 Bass kernel for per-sample Brownian-distance-covariance (BDC) pooling.

Problem: x [128, 640, 100] f32, t [1,1] f32 (log temperature).
  per sample: G = x @ x^T; dcov = d_i + d_j - 2G; dcov = max(dcov, 1e-4);
  z = sqrt(exp(t)*dcov + 1e-5); out = z - rowmean - colmean + totmean.
Output: [128, 409600] f32 (f16 on the wire, host upcasts).

Strategy (8 NeuronCores, pure data parallel, 16 samples/core):
  - Row-natural input layout "(r p)": partition p of chunk r holds x row
    r*128+p, so gram columns come out in natural order and every z-path
    AP is packed (enables DVE 2x/4x perf modes).
  - d = ||x_i||^2 via Pool-engine squares + DVE segmented reduce, batched
    over the 2-sample DMA groups.
  - Gram via TensorE; d_j enters the same PSUM accumulation through
    constant one-hot "selector" matmuls over the transposed bf16 hi/lo
    split; d_i enters via the per-partition activation bias, which also
    compensates bf16 rounding exactly on the diagonal (no clamp needed).
  - z = sqrt(...) computed per row-chunk on the Activation engine with
    accum_out giving per-chunk rowsums for free.
  - Double centering: per chunk, a 4x-mode tensor_scalar (z - s0_r) and a
    2x-mode tensor_sub against the f16 colmean broadcast (copied from the
    selector-matmul PSUM by the Pool engine).  Output staged f16 and
    DMA'd in two pieces per sample; host converts to f32.
  - Head/tail scalar chains batched across the 2-sample input groups to
    halve the DVE instruction count.
"""
import numpy as np
from contextlib import ExitStack

import concourse.bass as bass
import concourse.bacc as bacc
import concourse.tile as tile
from concourse import mybir
from concourse.bass_utils import run_bass_kernel_spmd

F32 = mybir.dt.float32
BF16 = mybir.dt.bfloat16
F16 = mybir.dt.float16
AF = mybir.ActivationFunctionType
OP = mybir.AluOpType

N_CORES = 8
B_TOTAL = 128
B_CORE = B_TOTAL // N_CORES  # 16
DIM = 640
M = 100
NCHUNK = DIM // 128  # 5
GSZ = 2
NG = B_CORE // GSZ  # 8 groups

_cached_nc = None


def build():
    nc = bacc.Bacc("TRN2", target_bir_lowering=False)
    x = nc.dram_tensor("x", [B_CORE, DIM, M], F32, kind="ExternalInput")
    consts = nc.dram_tensor("consts", [128, 2], F32, kind="ExternalInput")
    ident_in = nc.dram_tensor("ident", [128, 128], BF16, kind="ExternalInput")
    sel_in = nc.dram_tensor(
        "sel", [32 + 2 * NCHUNK, NCHUNK * 128], BF16, kind="ExternalInput"
    )
    out = nc.dram_tensor("out", [B_CORE, DIM * DIM], F16, kind="ExternalOutput")

    with tile.TileContext(nc) as tc, ExitStack() as ctx:
        const_p = ctx.enter_context(tc.tile_pool(name="const", bufs=1))
        xbp = ctx.enter_context(tc.tile_pool(name="xbp", bufs=4))
        sqp = ctx.enter_context(tc.tile_pool(name="sqp", bufs=4))
        hp = ctx.enter_context(tc.tile_pool(name="hp", bufs=4))
        xtp = ctx.enter_context(tc.tile_pool(name="xtp", bufs=5))
        zp = ctx.enter_context(tc.tile_pool(name="zp", bufs=6))
        opool = ctx.enter_context(tc.tile_pool(name="op", bufs=3))
        psamp = ctx.enter_context(tc.tile_pool(name="psamp", bufs=4))
        ps_g = ctx.enter_context(tc.tile_pool(name="psg", bufs=2, space="PSUM"))
        ps_m = ctx.enter_context(tc.tile_pool(name="psm", bufs=1, space="PSUM"))
        ps_x = ctx.enter_context(tc.tile_pool(name="psx", bufs=2, space="PSUM"))

        # ---- input prefetch first so sample 0's chain starts ASAP ----
        def in_dma(g, split=False):
            b0 = GSZ * g
            xbg = xbp.tile([128, GSZ, NCHUNK, M], BF16, tag="xb")
            if split:
                # per-sample pieces so sample b0's chain starts earlier
                for s in range(GSZ):
                    nc.gpsimd.dma_start(
                        xbg[:, s : s + 1],
                        x[b0 + s : b0 + s + 1].rearrange(
                            "s (r p) m -> p s r m", p=128
                        ),
                    )
            else:
                nc.gpsimd.dma_start(
                    xbg[:],
                    x[b0 : b0 + GSZ].rearrange("s (r p) m -> p s r m", p=128),
                )
            return xbg

        with tc.high_priority():
            xbg0 = in_dma(0, split=True)

        # ---- constants ----
        c_consts = const_p.tile([128, 2], F32)
        nc.sync.dma_start(c_consts[:], consts[:])
        neg2alpha = c_consts[:, 0:1]
        twoalpha = c_consts[:, 1:2]

        c_ident = const_p.tile([128, 128], BF16)
        nc.sync.dma_start(c_ident[:], ident_in[:])

        c_ones128 = const_p.tile([128, 128], F32)
        nc.vector.memset(c_ones128[:], 1.0)
        atl_warm = const_p.tile([1, 1], F32)
        nc.scalar.activation(atl_warm[:], c_ones128[0:1, 0:1], AF.Sqrt)
        # selector weights: SEL_j = c_sel[:, j*128:(j+1)*128] is [2*NCHUNK,128]
        # with ones in rows j and NCHUNK+j -> matmul broadcasts (hi+lo) row j
        # of a [2*NCHUNK,128] tile across all 128 output partitions.

        def emit_sq_pair(g, xbg, eng=None, split=False):
            sqs = sqp.tile([128, GSZ, NCHUNK * M], F32, tag="sq")
            xv = xbg[:].rearrange("p s r m -> p s (r m)")
            e = eng or nc.gpsimd
            if split:
                for s in range(GSZ):
                    e.tensor_mul(
                        sqs[:, s : s + 1], xv[:, s : s + 1], xv[:, s : s + 1]
                    )
            else:
                e.tensor_mul(sqs[:], xv, xv)
            return sqs

        def head_pair(g, xbg, sqs, xt_eng=None):
            """Group head: xT transposes/copies + batched d/hi-lo/bias/t5."""
            xTs = []
            for s in range(GSZ):
                xps = ps_x.tile([M, DIM], BF16, tag="xps")
                for r in range(NCHUNK):
                    nc.tensor.transpose(
                        xps[:, r * 128 : (r + 1) * 128], xbg[:, s, r, :],
                        c_ident[:],
                    )
                xT = xtp.tile([M, DIM], BF16, tag="xT")
                if xt_eng is nc.scalar:
                    nc.scalar.copy(xT[:], xps[:])
                else:
                    (xt_eng or nc.vector).tensor_copy(xT[:], xps[:])
                xTs.append(xT)
            d2 = hp.tile([128, GSZ, NCHUNK], F32, tag="d")
            nc.vector.tensor_reduce(
                d2[:].rearrange("p s r -> p (s r)"),
                sqs[:].rearrange("p s (r m) -> p (s r) m", r=NCHUNK),
                axis=mybir.AxisListType.X, op=OP.add,
            )
            # hi/lo split of -0.5*d, both samples at once
            hstack2 = hp.tile([128, GSZ, 2 * NCHUNK], BF16, tag="hstack")
            hi = hstack2[:, :, 0:NCHUNK]
            lo = hstack2[:, :, NCHUNK : 2 * NCHUNK]
            nc.vector.tensor_scalar(
                out=hi, in0=d2[:], scalar1=-0.5, scalar2=None, op0=OP.mult,
            )
            hres2 = hp.tile([128, GSZ, NCHUNK], F32, tag="hres")
            nc.vector.tensor_scalar(
                out=hres2[:], in0=d2[:], scalar1=-0.5, scalar2=None, op0=OP.mult
            )
            nc.vector.tensor_sub(lo, hres2[:], hi)
            # transpose hi/lo stacks -> t5pair rows [0:10] and [32:42]
            # (matmul operands need base partition 0/32/64)
            xps2 = ps_x.tile([M, DIM], BF16, tag="xps")
            t5pair = hp.tile([32 + 2 * NCHUNK, 128], BF16, tag="t5")
            for s in range(GSZ):
                nc.tensor.transpose(
                    xps2[32 * s : 32 * s + 2 * NCHUNK, 0:128],
                    hstack2[:, s, :], c_ident[:]
                )
                nc.vector.tensor_copy(
                    t5pair[32 * s : 32 * s + 2 * NCHUNK, :],
                    xps2[32 * s : 32 * s + 2 * NCHUNK, 0:128],
                )
            # bias = 2a*(d + hi + lo) + 1e-5 (exact diagonal compensation)
            tmpb2 = hp.tile([128, GSZ, NCHUNK], F32, tag="tmpb")
            nc.vector.tensor_add(tmpb2[:], d2[:], hi)
            nc.vector.tensor_add(tmpb2[:], tmpb2[:], lo)
            bias2 = hp.tile([128, GSZ, NCHUNK], F32, tag="bias")
            nc.vector.tensor_scalar(
                out=bias2[:], in0=tmpb2[:], scalar1=twoalpha, scalar2=1e-5,
                op0=OP.mult, op1=OP.add,
            )
            rowsum2 = hp.tile([128, GSZ, NCHUNK], F32, tag="rowsum")
            return {
                2 * g + s: (
                    xTs[s],
                    bias2[:, s],
                    t5pair[32 * s : 32 * s + 2 * NCHUNK, :],
                    rowsum2[:, s],
                    rowsum2,
                )
                for s in range(GSZ)
            }

        def head_pair_split(g, xbg):
            """Warmup head for group 0: fully per-sample chains (including
            squares) so sample 0's bias/t5/xT are ready as early as possible."""
            sqs = sqp.tile([128, GSZ, NCHUNK * M], F32, tag="sq")
            xv = xbg[:].rearrange("p s r m -> p s (r m)")
            d2 = hp.tile([128, GSZ, NCHUNK], F32, tag="d")
            hstack2 = hp.tile([128, GSZ, 2 * NCHUNK], BF16, tag="hstack")
            tmpb2 = hp.tile([128, GSZ, NCHUNK], F32, tag="tmpb")
            bias2 = hp.tile([128, GSZ, NCHUNK], F32, tag="bias")
            hres2 = hp.tile([128, GSZ, NCHUNK], F32, tag="hres")
            t5pair = hp.tile([32 + 2 * NCHUNK, 128], BF16, tag="t5")
            rowsum2 = hp.tile([128, GSZ, NCHUNK], F32, tag="rowsum")
            xTs = []
            for s in range(GSZ):
                nc.vector.tensor_mul(
                    sqs[:, s : s + 1], xv[:, s : s + 1], xv[:, s : s + 1]
                )
                xps = ps_x.tile([M, DIM], BF16, tag="xps")
                for r in range(NCHUNK):
                    nc.tensor.transpose(
                        xps[:, r * 128 : (r + 1) * 128], xbg[:, s, r, :],
                        c_ident[:],
                    )
                xT = xtp.tile([M, DIM], BF16, tag="xT")
                nc.vector.tensor_copy(xT[:], xps[:])
                xTs.append(xT)
                hi = hstack2[:, s, 0:NCHUNK]
                lo = hstack2[:, s, NCHUNK : 2 * NCHUNK]
                nc.vector.tensor_reduce(
                    d2[:, s],
                    sqs[:, s].rearrange("p (r m) -> p r m", r=NCHUNK),
                    axis=mybir.AxisListType.X, op=OP.add,
                )
                nc.vector.tensor_scalar(
                    out=hi, in0=d2[:, s], scalar1=-0.5, scalar2=None,
                    op0=OP.mult,
                )
                nc.vector.tensor_scalar(
                    out=hres2[:, s], in0=d2[:, s], scalar1=-0.5, scalar2=None,
                    op0=OP.mult,
                )
                nc.vector.tensor_sub(lo, hres2[:, s], hi)
                xps2 = ps_x.tile([M, DIM], BF16, tag="xps")
                nc.tensor.transpose(
                    xps2[32 * s : 32 * s + 2 * NCHUNK, 0:128],
                    hstack2[:, s, :], c_ident[:]
                )
                nc.vector.tensor_copy(
                    t5pair[32 * s : 32 * s + 2 * NCHUNK, :],
                    xps2[32 * s : 32 * s + 2 * NCHUNK, 0:128],
                )
                nc.vector.tensor_add(tmpb2[:, s], d2[:, s], hi)
                nc.vector.tensor_add(tmpb2[:, s], tmpb2[:, s], lo)
                nc.vector.tensor_scalar(
                    out=bias2[:, s], in0=tmpb2[:, s], scalar1=twoalpha,
                    scalar2=1e-5, op0=OP.mult, op1=OP.add,
                )
            return {
                2 * g + s: (
                    xTs[s],
                    bias2[:, s],
                    t5pair[32 * s : 32 * s + 2 * NCHUNK, :],
                    rowsum2[:, s],
                    rowsum2,
                )
                for s in range(GSZ)
            }

        def emit_sample_c(n, head, z):
            xT, bias_s, t5, rowsum_s, _ = head
            sb = 32 * (n % GSZ)
            sel_slice = c_sel[sb : sb + 2 * NCHUNK, :]
            for r in range(NCHUNK):
                lhsT = xT[:, r * 128 : (r + 1) * 128]
                ps = ps_g.tile([128, DIM], F32, tag="gram")
                nc.tensor.matmul(
                    ps[:, 0:512], lhsT, xT[:, 0:512],
                    start=True, stop=False, skip_group_check=True,
                )
                nc.tensor.matmul(
                    ps[:, 512:640], lhsT, xT[:, 512:640],
                    start=True, stop=False, skip_group_check=True,
                )
                for j in range(NCHUNK):
                    nc.tensor.matmul(
                        ps[:, j * 128 : (j + 1) * 128],
                        sel_slice[:, j * 128 : (j + 1) * 128], t5,
                        start=False, stop=True, skip_group_check=True,
                    )
                nc.scalar.activation(
                    z[:, r, :], ps[:], AF.Sqrt,
                    bias=bias_s[:, r : r + 1],
                    scale=neg2alpha,
                    accum_out=rowsum_s[:, r : r + 1],
                )

        def tail_pair(g, sts, drain=False):
            """Batched tail for samples 2g, 2g+1: means chain + centering."""
            st_a, st_b = sts[2 * g], sts[2 * g + 1]
            rowsum2 = st_a["rowsum_pair"][:]
            rs2 = psamp.tile([128, GSZ], F32, tag="rs2")
            nc.vector.tensor_reduce(
                rs2[:], rowsum2, axis=mybir.AxisListType.X, op=OP.add
            )
            rm2 = psamp.tile([128, GSZ, NCHUNK], F32, tag="rm2")
            nc.vector.tensor_scalar(
                out=rm2[:], in0=rowsum2, scalar1=1.0 / DIM, scalar2=None,
                op0=OP.mult,
            )
            rmstack2 = psamp.tile([128, GSZ, 2 * NCHUNK], BF16, tag="rmstack")
            rhi = rmstack2[:, :, 0:NCHUNK]
            rlo = rmstack2[:, :, NCHUNK : 2 * NCHUNK]
            nc.vector.tensor_copy(rhi, rm2[:])
            nc.vector.tensor_sub(rlo, rm2[:], rhi)
            xps3 = ps_x.tile([M, DIM], BF16, tag="xps")
            t10pair = psamp.tile([32 + 2 * NCHUNK, 128], BF16, tag="t10")
            for s in range(GSZ):
                nc.tensor.transpose(
                    xps3[32 * s : 32 * s + 2 * NCHUNK, 0:128],
                    rmstack2[:, s, :], c_ident[:]
                )
                nc.vector.tensor_copy(
                    t10pair[32 * s : 32 * s + 2 * NCHUNK, :],
                    xps3[32 * s : 32 * s + 2 * NCHUNK, 0:128],
                )

            for s, st in ((0, st_a), (1, st_b)):
                b, z = st["n"], st["z"]
                mps = ps_m.tile([128, DIM], F32, tag="mps")
                if s == 0:
                    nc.tensor.matmul(
                        mps[:, 0:GSZ], c_ones128[:], rs2[:],
                        start=True, stop=True, skip_group_check=True,
                    )
                    tm2 = psamp.tile([128, GSZ], F32, tag="tm2")
                    nc.vector.tensor_scalar(
                        out=tm2[:], in0=mps[:, 0:GSZ],
                        scalar1=1.0 / (DIM * DIM), scalar2=None, op0=OP.mult,
                    )
                    st_a["tm2"] = tm2
                tm2 = st_a["tm2"]
                s0_b = psamp.tile([128, NCHUNK], F32, tag="s0")
                nc.vector.tensor_scalar(
                    out=s0_b[:], in0=rm2[:, s], scalar1=tm2[:, s : s + 1],
                    scalar2=None, op0=OP.subtract,
                )
                sel_slice = c_sel[32 * s : 32 * s + 2 * NCHUNK, :]
                for j in range(NCHUNK):
                    nc.tensor.matmul(
                        mps[:, j * 128 : (j + 1) * 128],
                        sel_slice[:, j * 128 : (j + 1) * 128],
                        t10pair[32 * s : 32 * s + 2 * NCHUNK, :],
                        start=True, stop=True, skip_group_check=True,
                    )
                mv16 = psamp.tile([128, DIM], F16, tag="mv16")
                nc.vector.tensor_scalar(
                    out=mv16[:], in0=mps[:], scalar1=1.0, scalar2=None,
                    op0=OP.mult,
                )
                outt = opool.tile([128, NCHUNK, DIM], F16, tag="outt")
                for r in range(NCHUNK):
                    eng = nc.gpsimd if r < 3 else nc.vector
                    eng.tensor_scalar(
                        out=outt[:, r, :], in0=z[:, r, :],
                        scalar1=s0_b[:, r : r + 1], scalar2=None,
                        op0=OP.subtract,
                    )
                    nc.vector.tensor_sub(outt[:, r, :], outt[:, r, :], mv16[:])
                    if r == 2:
                        nc.sync.dma_start(
                            out[b].rearrange(
                                "(r p c) -> p r c", r=NCHUNK, p=128
                            )[:, 0:3, :],
                            outt[:, 0:3, :],
                        )
                nc.sync.dma_start(
                    out[b].rearrange("(r p c) -> p r c", r=NCHUNK, p=128)[:, 3:5, :],
                    outt[:, 3:5, :],
                )

        def tail_single_stats(n, sts, last=False):
            """Per-sample tail stats for the drain: means chain + colmean
            broadcast, emitted eagerly so it overlaps the final acts."""
            st = sts[n]
            s = n % 2
            rowsum2 = st["rowsum_pair"][:]
            rm_s = psamp.tile([128, NCHUNK], F32, tag="rm1")
            rs_acc = psamp.tile([128, 1], F32, tag="rs1")
            nc.vector.tensor_scalar(
                out=rm_s[:], in0=rowsum2[:, s], scalar1=1.0 / DIM, scalar2=0.0,
                op0=OP.mult, op1=OP.add, accum_out=rs_acc[:],
            )
            rmstack = psamp.tile([128, 2 * NCHUNK], BF16, tag="rmst1")
            nc.vector.tensor_copy(rmstack[:, 0:NCHUNK], rm_s[:])
            nc.vector.tensor_sub(
                rmstack[:, NCHUNK : 2 * NCHUNK], rm_s[:], rmstack[:, 0:NCHUNK]
            )
            xps3 = ps_x.tile([M, DIM], BF16, tag="xps")
            nc.tensor.transpose(
                xps3[0 : 2 * NCHUNK, 0:128], rmstack[:], c_ident[:]
            )
            t10 = psamp.tile([2 * NCHUNK, 128], BF16, tag="t10s")
            nc.vector.tensor_copy(t10[:], xps3[0 : 2 * NCHUNK, 0:128])
            mps = ps_m.tile([128, DIM], F32, tag="mps")
            nc.tensor.matmul(
                mps[:, 0:1], c_ones128[:], rs_acc[:],
                start=True, stop=True, skip_group_check=True,
            )
            tm_b = psamp.tile([128, 1], F32, tag="tm1")
            nc.vector.tensor_scalar(
                out=tm_b[:], in0=mps[:, 0:1], scalar1=1.0 / DIM,
                scalar2=None, op0=OP.mult,
            )
            s0_b = psamp.tile([128, NCHUNK], F32, tag="s0")
            nc.vector.tensor_scalar(
                out=s0_b[:], in0=rm_s[:], scalar1=tm_b[:], scalar2=None,
                op0=OP.subtract,
            )
            for j in range(NCHUNK):
                nc.tensor.matmul(
                    mps[:, j * 128 : (j + 1) * 128],
                    c_sel[0 : 2 * NCHUNK, j * 128 : (j + 1) * 128], t10[:],
                    start=True, stop=True, skip_group_check=True,
                )
            mv16 = psamp.tile([128, DIM], F16, tag="mv16")
            if last:
                nc.scalar.copy(mv16[:], mps[:])
            else:
                nc.vector.tensor_scalar(
                    out=mv16[:], in0=mps[:], scalar1=1.0, scalar2=None,
                    op0=OP.mult,
                )
            st["s0s"], st["mv16s"] = s0_b, mv16

        def tail_single_z(n, sts, last=False):
            st = sts[n]
            s0_b, mv16 = st["s0s"], st["mv16s"]
            b, z = st["n"], st["z"]
            outt = opool.tile([128, NCHUNK, DIM], F16, tag="outt")
            for r in range(NCHUNK):
                eng = nc.vector if last else (nc.gpsimd if r < 3 else nc.vector)
                eng.tensor_scalar(
                    out=outt[:, r, :], in0=z[:, r, :],
                    scalar1=s0_b[:, r : r + 1], scalar2=None,
                    op0=OP.subtract,
                )
                nc.vector.tensor_sub(outt[:, r, :], outt[:, r, :], mv16[:])
                if r == 2:
                    nc.sync.dma_start(
                        out[b].rearrange(
                            "(r p c) -> p r c", r=NCHUNK, p=128
                        )[:, 0:3, :],
                        outt[:, 0:3, :],
                    )
            nc.sync.dma_start(
                out[b].rearrange("(r p c) -> p r c", r=NCHUNK, p=128)[:, 3:5, :],
                outt[:, 3:5, :],
            )

        # ---- software pipeline over 2-sample groups ----
        # Warmup squares/xT copies ride the then-idle DVE so the Pool engine
        # (DMA gen + steady-state squares/copies) isn't the ramp bottleneck.
        xbgs = {0: xbg0}
        c_sel = const_p.tile([32 + 2 * NCHUNK, NCHUNK * 128], BF16)
        nc.sync.dma_start(c_sel[:], sel_in[:])
        sq2s = {}
        heads = {}
        heads.update(head_pair_split(0, xbgs[0]))
        xbgs[1] = in_dma(1)
        sq2s[1] = emit_sq_pair(1, xbgs[1], eng=nc.vector)
        heads.update(head_pair(1, xbgs[1], sq2s.pop(1), xt_eng=nc.vector))
        xbgs[2] = in_dma(2)
        sq2s[2] = emit_sq_pair(2, xbgs[2])
        sts = {}
        for n in range(B_CORE):
            g = n // 2
            z = zp.tile([128, NCHUNK, DIM], F16, tag="z")
            head = heads.pop(n)
            emit_sample_c(n, head, z)
            sts[n] = {"n": n, "z": z, "rowsum_pair": head[4]}
            if n % 2 == 0:
                if g + 2 < NG:
                    heads.update(head_pair(g + 2, xbgs[g + 2], sq2s.pop(g + 2)))
                if g + 3 < NG:
                    xbgs[g + 3] = in_dma(g + 3)
                    sq2s[g + 3] = emit_sq_pair(g + 3, xbgs[g + 3])
            elif g >= 1:
                tail_pair(g - 1, sts)
        tail_single_stats(B_CORE - 2, sts)
        tail_single_stats(B_CORE - 1, sts, last=True)
        tail_single_z(B_CORE - 2, sts)
        tail_single_z(B_CORE - 1, sts, last=True)

    nc.compile()
    return nc


def _get_nc():
    global _cached_nc
    if _cached_nc is None:
        _cached_nc = build()
    return _cached_nc


def make_in_maps(x: np.ndarray, t: np.ndarray):
    import ml_dtypes

    alpha = float(np.exp(t.astype(np.float64))[0, 0])
    consts = np.zeros((128, 2), dtype=np.float32)
    consts[:, 0] = -2.0 * alpha
    consts[:, 1] = 2.0 * alpha
    ident = np.eye(128, dtype=ml_dtypes.bfloat16)
    sel = np.zeros((32 + 2 * NCHUNK, NCHUNK * 128), dtype=ml_dtypes.bfloat16)
    for base in (0, 32):
        for j in range(NCHUNK):
            sel[base + j, j * 128 : (j + 1) * 128] = 1.0
            sel[base + NCHUNK + j, j * 128 : (j + 1) * 128] = 1.0
    xs = x.reshape(N_CORES, B_CORE, DIM, M)
    return [
        {"x": np.ascontiguousarray(xs[c]), "consts": consts, "ident": ident,
         "sel": sel}
        for c in range(N_CORES)
    ]


def kernel(x: np.ndarray, t: np.ndarray) -> np.ndarray:
    x = np.asarray(x, dtype=np.float32)
    t = np.asarray(t, dtype=np.float32)
    nc = _get_nc()
    res = run_bass_kernel_spmd(nc, make_in_maps(x, t), core_ids=list(range(N_CORES)))
    return np.concatenate(
        [r["out"].astype(np.float32) for r in res.results], axis=0
    )


# revision 78
# speedup vs baseline: 1.0019x; 1.0019x over previous
"""Trainium2 Bass kernel for per-sample Brownian-distance-covariance (BDC) pooling.

Problem: x [128, 640, 100] f32, t [1,1] f32 (log temperature).
  per sample: G = x @ x^T; dcov = d_i + d_j - 2G; dcov = max(dcov, 1e-4);
  z = sqrt(exp(t)*dcov + 1e-5); out = z - rowmean - colmean + totmean.
Output: [128, 409600] f32 (f16 on the wire, host upcasts).

Strategy (8 NeuronCores, pure data parallel, 16 samples/core):
  - Row-natural input layout "(r p)": partition p of chunk r holds x row
    r*128+p, so gram columns come out in natural order and every z-path
    AP is packed (enables DVE 2x/4x perf modes).
  - d = ||x_i||^2 via Pool-engine squares + DVE segmented reduce, batched
    over the 2-sample DMA groups.
  - Gram via TensorE; d_j enters the same PSUM accumulation through
    constant one-hot "selector" matmuls over the transposed bf16 hi/lo
    split; d_i enters via the per-partition activation bias, which also
    compensates bf16 rounding exactly on the diagonal (no clamp needed).
  - z = sqrt(...) computed per row-chunk on the Activation engine with
    accum_out giving per-chunk rowsums for free.
  - Double centering: per chunk, a 4x-mode tensor_scalar (z - s0_r) and a
    2x-mode tensor_sub against the f16 colmean broadcast (copied from the
    selector-matmul PSUM by the Pool engine).  Output staged f16 and
    DMA'd in two pieces per sample; host converts to f32.
  - Head/tail scalar chains batched across the 2-sample input groups to
    halve the DVE instruction count.
"""
import numpy as np
from contextlib import ExitStack

import concourse.bass as bass
import concourse.bacc as bacc
import concourse.tile as tile
from concourse import mybir
from concourse.bass_utils import run_bass_kernel_spmd

F32 = mybir.dt.float32
BF16 = mybir.dt.bfloat16
F16 = mybir.dt.float16
AF = mybir.ActivationFunctionType
OP = mybir.AluOpType

N_CORES = 8
B_TOTAL = 128
B_CORE = B_TOTAL // N_CORES  # 16
DIM = 640
M = 100
NCHUNK = DIM // 128  # 5
GSZ = 2
NG = B_CORE // GSZ  # 8 groups

_cached_nc = None


def build():
    nc = bacc.Bacc("TRN2", target_bir_lowering=False)
    x = nc.dram_tensor("x", [B_CORE, DIM, M], F32, kind="ExternalInput")
    consts = nc.dram_tensor("consts", [128, 2], F32, kind="ExternalInput")
    ident_in = nc.dram_tensor("ident", [128, 128], BF16, kind="ExternalInput")
    sel_in = nc.dram_tensor(
        "sel", [32 + 2 * NCHUNK, NCHUNK * 128], BF16, kind="ExternalInput"
    )
    out = nc.dram_tensor("out", [B_CORE, DIM * DIM], F16, kind="ExternalOutput")

    with tile.TileContext(nc) as tc, ExitStack() as ctx:
        const_p = ctx.enter_context(tc.tile_pool(name="const", bufs=1))
        xbp = ctx.enter_context(tc.tile_pool(name="xbp", bufs=5))
        sqp = ctx.enter_context(tc.tile_pool(name="sqp", bufs=5))
        hp = ctx.enter_context(tc.tile_pool(name="hp", bufs=4))
        xtp = ctx.enter_context(tc.tile_pool(name="xtp", bufs=5))
        zp = ctx.enter_context(tc.tile_pool(name="zp", bufs=6))
        opool = ctx.enter_context(tc.tile_pool(name="op", bufs=3))
        psamp = ctx.enter_context(tc.tile_pool(name="psamp", bufs=4))
        ps_g = ctx.enter_context(tc.tile_pool(name="psg", bufs=2, space="PSUM"))
        ps_m = ctx.enter_context(tc.tile_pool(name="psm", bufs=1, space="PSUM"))
        ps_x = ctx.enter_context(tc.tile_pool(name="psx", bufs=2, space="PSUM"))

        # ---- input prefetch first so sample 0's chain starts ASAP ----
        def in_dma(g, split=False):
            b0 = GSZ * g
            xbg = xbp.tile([128, GSZ, NCHUNK, M], BF16, tag="xb")
            if split:
                # per-sample pieces so sample b0's chain starts earlier
                for s in range(GSZ):
                    nc.gpsimd.dma_start(
                        xbg[:, s : s + 1],
                        x[b0 + s : b0 + s + 1].rearrange(
                            "s (r p) m -> p s r m", p=128
                        ),
                    )
            else:
                nc.gpsimd.dma_start(
                    xbg[:],
                    x[b0 : b0 + GSZ].rearrange("s (r p) m -> p s r m", p=128),
                )
            return xbg

        with tc.high_priority():
            xbg0 = in_dma(0, split=True)

        # ---- constants ----
        c_consts = const_p.tile([128, 2], F32)
        nc.sync.dma_start(c_consts[:], consts[:])
        neg2alpha = c_consts[:, 0:1]
        twoalpha = c_consts[:, 1:2]

        c_ident = const_p.tile([128, 128], BF16)
        nc.sync.dma_start(c_ident[:], ident_in[:])

        c_ones128 = const_p.tile([128, 128], F32)
        nc.vector.memset(c_ones128[:], 1.0)
        atl_warm = const_p.tile([1, 1], F32)
        nc.scalar.activation(atl_warm[:], c_ones128[0:1, 0:1], AF.Sqrt)
        # selector weights: SEL_j = c_sel[:, j*128:(j+1)*128] is [2*NCHUNK,128]
        # with ones in rows j and NCHUNK+j -> matmul broadcasts (hi+lo) row j
        # of a [2*NCHUNK,128] tile across all 128 output partitions.

        def emit_sq_pair(g, xbg, eng=None, split=False):
            sqs = sqp.tile([128, GSZ, NCHUNK * M], F32, tag="sq")
            xv = xbg[:].rearrange("p s r m -> p s (r m)")
            e = eng or nc.gpsimd
            if split:
                for s in range(GSZ):
                    e.tensor_mul(
                        sqs[:, s : s + 1], xv[:, s : s + 1], xv[:, s : s + 1]
                    )
            else:
                e.tensor_mul(sqs[:], xv, xv)
            return sqs

        def head_pair(g, xbg, sqs, xt_eng=None):
            """Group head: xT transposes/copies + batched d/hi-lo/bias/t5."""
            xTs = []
            for s in range(GSZ):
                xps = ps_x.tile([M, DIM], BF16, tag="xps")
                for r in range(NCHUNK):
                    nc.tensor.transpose(
                        xps[:, r * 128 : (r + 1) * 128], xbg[:, s, r, :],
                        c_ident[:],
                    )
                xT = xtp.tile([M, DIM], BF16, tag="xT")
                if xt_eng is nc.scalar:
                    nc.scalar.copy(xT[:], xps[:])
                else:
                    (xt_eng or nc.vector).tensor_copy(xT[:], xps[:])
                xTs.append(xT)
            d2 = hp.tile([128, GSZ, NCHUNK], F32, tag="d")
            nc.vector.tensor_reduce(
                d2[:].rearrange("p s r -> p (s r)"),
                sqs[:].rearrange("p s (r m) -> p (s r) m", r=NCHUNK),
                axis=mybir.AxisListType.X, op=OP.add,
            )
            # hi/lo split of -0.5*d, both samples at once
            hstack2 = hp.tile([128, GSZ, 2 * NCHUNK], BF16, tag="hstack")
            hi = hstack2[:, :, 0:NCHUNK]
            lo = hstack2[:, :, NCHUNK : 2 * NCHUNK]
            nc.vector.tensor_scalar(
                out=hi, in0=d2[:], scalar1=-0.5, scalar2=None, op0=OP.mult,
            )
            hres2 = hp.tile([128, GSZ, NCHUNK], F32, tag="hres")
            nc.vector.tensor_scalar(
                out=hres2[:], in0=d2[:], scalar1=-0.5, scalar2=None, op0=OP.mult
            )
            nc.vector.tensor_sub(lo, hres2[:], hi)
            # transpose hi/lo stacks -> t5pair rows [0:10] and [32:42]
            # (matmul operands need base partition 0/32/64)
            xps2 = ps_x.tile([M, DIM], BF16, tag="xps")
            t5pair = hp.tile([32 + 2 * NCHUNK, 128], BF16, tag="t5")
            for s in range(GSZ):
                nc.tensor.transpose(
                    xps2[32 * s : 32 * s + 2 * NCHUNK, 0:128],
                    hstack2[:, s, :], c_ident[:]
                )
                nc.vector.tensor_copy(
                    t5pair[32 * s : 32 * s + 2 * NCHUNK, :],
                    xps2[32 * s : 32 * s + 2 * NCHUNK, 0:128],
                )
            # bias = 2a*(d + hi + lo) + 1e-5 (exact diagonal compensation)
            tmpb2 = hp.tile([128, GSZ, NCHUNK], F32, tag="tmpb")
            nc.vector.tensor_add(tmpb2[:], d2[:], hi)
            nc.vector.tensor_add(tmpb2[:], tmpb2[:], lo)
            bias2 = hp.tile([128, GSZ, NCHUNK], F32, tag="bias")
            nc.vector.tensor_scalar(
                out=bias2[:], in0=tmpb2[:], scalar1=twoalpha, scalar2=1e-5,
                op0=OP.mult, op1=OP.add,
            )
            rowsum2 = hp.tile([128, GSZ, NCHUNK], F32, tag="rowsum")
            return {
                2 * g + s: (
                    xTs[s],
                    bias2[:, s],
                    t5pair[32 * s : 32 * s + 2 * NCHUNK, :],
                    rowsum2[:, s],
                    rowsum2,
                )
                for s in range(GSZ)
            }

        def head_pair_split(g, xbg):
            """Warmup head for group 0: fully per-sample chains (including
            squares) so sample 0's bias/t5/xT are ready as early as possible."""
            sqs = sqp.tile([128, GSZ, NCHUNK * M], F32, tag="sq")
            xv = xbg[:].rearrange("p s r m -> p s (r m)")
            d2 = hp.tile([128, GSZ, NCHUNK], F32, tag="d")
            hstack2 = hp.tile([128, GSZ, 2 * NCHUNK], BF16, tag="hstack")
            tmpb2 = hp.tile([128, GSZ, NCHUNK], F32, tag="tmpb")
            bias2 = hp.tile([128, GSZ, NCHUNK], F32, tag="bias")
            hres2 = hp.tile([128, GSZ, NCHUNK], F32, tag="hres")
            t5pair = hp.tile([32 + 2 * NCHUNK, 128], BF16, tag="t5")
            rowsum2 = hp.tile([128, GSZ, NCHUNK], F32, tag="rowsum")
            xTs = []
            for s in range(GSZ):
                nc.vector.tensor_mul(
                    sqs[:, s : s + 1], xv[:, s : s + 1], xv[:, s : s + 1]
                )
                xps = ps_x.tile([M, DIM], BF16, tag="xps")
                for r in range(NCHUNK):
                    nc.tensor.transpose(
                        xps[:, r * 128 : (r + 1) * 128], xbg[:, s, r, :],
                        c_ident[:],
                    )
                xT = xtp.tile([M, DIM], BF16, tag="xT")
                nc.vector.tensor_copy(xT[:], xps[:])
                xTs.append(xT)
                hi = hstack2[:, s, 0:NCHUNK]
                lo = hstack2[:, s, NCHUNK : 2 * NCHUNK]
                nc.vector.tensor_reduce(
                    d2[:, s],
                    sqs[:, s].rearrange("p (r m) -> p r m", r=NCHUNK),
                    axis=mybir.AxisListType.X, op=OP.add,
                )
                nc.vector.tensor_scalar(
                    out=hi, in0=d2[:, s], scalar1=-0.5, scalar2=None,
                    op0=OP.mult,
                )
                nc.vector.tensor_scalar(
                    out=hres2[:, s], in0=d2[:, s], scalar1=-0.5, scalar2=None,
                    op0=OP.mult,
                )
                nc.vector.tensor_sub(lo, hres2[:, s], hi)
                xps2 = ps_x.tile([M, DIM], BF16, tag="xps")
                nc.tensor.transpose(
                    xps2[32 * s : 32 * s + 2 * NCHUNK, 0:128],
                    hstack2[:, s, :], c_ident[:]
                )
                nc.vector.tensor_copy(
                    t5pair[32 * s : 32 * s + 2 * NCHUNK, :],
                    xps2[32 * s : 32 * s + 2 * NCHUNK, 0:128],
                )
                nc.vector.tensor_add(tmpb2[:, s], d2[:, s], hi)
                nc.vector.tensor_add(tmpb2[:, s], tmpb2[:, s], lo)
                nc.vector.tensor_scalar(
                    out=bias2[:, s], in0=tmpb2[:, s], scalar1=twoalpha,
                    scalar2=1e-5, op0=OP.mult, op1=OP.add,
                )
            return {
                2 * g + s: (
                    xTs[s],
                    bias2[:, s],
                    t5pair[32 * s : 32 * s + 2 * NCHUNK, :],
                    rowsum2[:, s],
                    rowsum2,
                )
                for s in range(GSZ)
            }

        def emit_sample_c(n, head, z):
            xT, bias_s, t5, rowsum_s, _ = head
            sb = 32 * (n % GSZ)
            sel_slice = c_sel[sb : sb + 2 * NCHUNK, :]
            for r in range(NCHUNK):
                lhsT = xT[:, r * 128 : (r + 1) * 128]
                ps = ps_g.tile([128, DIM], F32, tag="gram")
                nc.tensor.matmul(
                    ps[:, 0:512], lhsT, xT[:, 0:512],
                    start=True, stop=False, skip_group_check=True,
                )
                nc.tensor.matmul(
                    ps[:, 512:640], lhsT, xT[:, 512:640],
                    start=True, stop=False, skip_group_check=True,
                )
                for j in range(NCHUNK):
                    nc.tensor.matmul(
                        ps[:, j * 128 : (j + 1) * 128],
                        sel_slice[:, j * 128 : (j + 1) * 128], t5,
                        start=False, stop=True, skip_group_check=True,
                    )
                nc.scalar.activation(
                    z[:, r, :], ps[:], AF.Sqrt,
                    bias=bias_s[:, r : r + 1],
                    scale=neg2alpha,
                    accum_out=rowsum_s[:, r : r + 1],
                )

        def tail_pair(g, sts, drain=False):
            """Batched tail for samples 2g, 2g+1: means chain + centering."""
            st_a, st_b = sts[2 * g], sts[2 * g + 1]
            rowsum2 = st_a["rowsum_pair"][:]
            rs2 = psamp.tile([128, GSZ], F32, tag="rs2")
            nc.vector.tensor_reduce(
                rs2[:], rowsum2, axis=mybir.AxisListType.X, op=OP.add
            )
            rm2 = psamp.tile([128, GSZ, NCHUNK], F32, tag="rm2")
            nc.vector.tensor_scalar(
                out=rm2[:], in0=rowsum2, scalar1=1.0 / DIM, scalar2=None,
                op0=OP.mult,
            )
            rmstack2 = psamp.tile([128, GSZ, 2 * NCHUNK], BF16, tag="rmstack")
            rhi = rmstack2[:, :, 0:NCHUNK]
            rlo = rmstack2[:, :, NCHUNK : 2 * NCHUNK]
            nc.vector.tensor_copy(rhi, rm2[:])
            nc.vector.tensor_sub(rlo, rm2[:], rhi)
            xps3 = ps_x.tile([M, DIM], BF16, tag="xps")
            t10pair = psamp.tile([32 + 2 * NCHUNK, 128], BF16, tag="t10")
            for s in range(GSZ):
                nc.tensor.transpose(
                    xps3[32 * s : 32 * s + 2 * NCHUNK, 0:128],
                    rmstack2[:, s, :], c_ident[:]
                )
                nc.vector.tensor_copy(
                    t10pair[32 * s : 32 * s + 2 * NCHUNK, :],
                    xps3[32 * s : 32 * s + 2 * NCHUNK, 0:128],
                )

            for s, st in ((0, st_a), (1, st_b)):
                b, z = st["n"], st["z"]
                mps = ps_m.tile([128, DIM], F32, tag="mps")
                if s == 0:
                    nc.tensor.matmul(
                        mps[:, 0:GSZ], c_ones128[:], rs2[:],
                        start=True, stop=True, skip_group_check=True,
                    )
                    tm2 = psamp.tile([128, GSZ], F32, tag="tm2")
                    nc.vector.tensor_scalar(
                        out=tm2[:], in0=mps[:, 0:GSZ],
                        scalar1=1.0 / (DIM * DIM), scalar2=None, op0=OP.mult,
                    )
                    st_a["tm2"] = tm2
                tm2 = st_a["tm2"]
                s0_b = psamp.tile([128, NCHUNK], F32, tag="s0")
                nc.vector.tensor_scalar(
                    out=s0_b[:], in0=rm2[:, s], scalar1=tm2[:, s : s + 1],
                    scalar2=None, op0=OP.subtract,
                )
                sel_slice = c_sel[32 * s : 32 * s + 2 * NCHUNK, :]
                for j in range(NCHUNK):
                    nc.tensor.matmul(
                        mps[:, j * 128 : (j + 1) * 128],
                        sel_slice[:, j * 128 : (j + 1) * 128],
                        t10pair[32 * s : 32 * s + 2 * NCHUNK, :],
                        start=True, stop=True, skip_group_check=True,
                    )
                mv16 = psamp.tile([128, DIM], F16, tag="mv16")
                nc.vector.tensor_scalar(
                    out=mv16[:], in0=mps[:], scalar1=1.0, scalar2=None,
                    op0=OP.mult,
                )
                outt = opool.tile([128, NCHUNK, DIM], F16, tag="outt")
                for r in range(NCHUNK):
                    eng = nc.gpsimd if r < 3 else nc.vector
                    eng.tensor_scalar(
                        out=outt[:, r, :], in0=z[:, r, :],
                        scalar1=s0_b[:, r : r + 1], scalar2=None,
                        op0=OP.subtract,
                    )
                    nc.vector.tensor_sub(outt[:, r, :], outt[:, r, :], mv16[:])
                    if r == 2:
                        nc.sync.dma_start(
                            out[b].rearrange(
                                "(r p c) -> p r c", r=NCHUNK, p=128
                            )[:, 0:3, :],
                            outt[:, 0:3, :],
                        )
                nc.sync.dma_start(
                    out[b].rearrange("(r p c) -> p r c", r=NCHUNK, p=128)[:, 3:5, :],
                    outt[:, 3:5, :],
                )

        def tail_single_stats(n, sts, last=False):
            """Per-sample tail stats for the drain: means chain + colmean
            broadcast, emitted eagerly so it overlaps the final acts."""
            st = sts[n]
            s = n % 2
            rowsum2 = st["rowsum_pair"][:]
            rm_s = psamp.tile([128, NCHUNK], F32, tag="rm1")
            rs_acc = psamp.tile([128, 1], F32, tag="rs1")
            nc.vector.tensor_scalar(
                out=rm_s[:], in0=rowsum2[:, s], scalar1=1.0 / DIM, scalar2=0.0,
                op0=OP.mult, op1=OP.add, accum_out=rs_acc[:],
            )
            rmstack = psamp.tile([128, 2 * NCHUNK], BF16, tag="rmst1")
            nc.vector.tensor_copy(rmstack[:, 0:NCHUNK], rm_s[:])
            nc.vector.tensor_sub(
                rmstack[:, NCHUNK : 2 * NCHUNK], rm_s[:], rmstack[:, 0:NCHUNK]
            )
            xps3 = ps_x.tile([M, DIM], BF16, tag="xps")
            nc.tensor.transpose(
                xps3[0 : 2 * NCHUNK, 0:128], rmstack[:], c_ident[:]
            )
            t10 = psamp.tile([2 * NCHUNK, 128], BF16, tag="t10s")
            nc.vector.tensor_copy(t10[:], xps3[0 : 2 * NCHUNK, 0:128])
            mps = ps_m.tile([128, DIM], F32, tag="mps")
            nc.tensor.matmul(
                mps[:, 0:1], c_ones128[:], rs_acc[:],
                start=True, stop=True, skip_group_check=True,
            )
            tm_b = psamp.tile([128, 1], F32, tag="tm1")
            nc.vector.tensor_scalar(
                out=tm_b[:], in0=mps[:, 0:1], scalar1=1.0 / DIM,
                scalar2=None, op0=OP.mult,
            )
            s0_b = psamp.tile([128, NCHUNK], F32, tag="s0")
            nc.vector.tensor_scalar(
                out=s0_b[:], in0=rm_s[:], scalar1=tm_b[:], scalar2=None,
                op0=OP.subtract,
            )
            for j in range(NCHUNK):
                nc.tensor.matmul(
                    mps[:, j * 128 : (j + 1) * 128],
                    c_sel[0 : 2 * NCHUNK, j * 128 : (j + 1) * 128], t10[:],
                    start=True, stop=True, skip_group_check=True,
                )
            mv16 = psamp.tile([128, DIM], F16, tag="mv16")
            if last:
                nc.scalar.copy(mv16[:], mps[:])
            else:
                nc.vector.tensor_scalar(
                    out=mv16[:], in0=mps[:], scalar1=1.0, scalar2=None,
                    op0=OP.mult,
                )
            st["s0s"], st["mv16s"] = s0_b, mv16

        def tail_single_z(n, sts, last=False):
            st = sts[n]
            s0_b, mv16 = st["s0s"], st["mv16s"]
            b, z = st["n"], st["z"]
            outt = opool.tile([128, NCHUNK, DIM], F16, tag="outt")
            for r in range(NCHUNK):
                eng = nc.vector if last else (nc.gpsimd if r < 3 else nc.vector)
                eng.tensor_scalar(
                    out=outt[:, r, :], in0=z[:, r, :],
                    scalar1=s0_b[:, r : r + 1], scalar2=None,
                    op0=OP.subtract,
                )
                nc.vector.tensor_sub(outt[:, r, :], outt[:, r, :], mv16[:])
                if r == 2:
                    nc.sync.dma_start(
                        out[b].rearrange(
                            "(r p c) -> p r c", r=NCHUNK, p=128
                        )[:, 0:3, :],
                        outt[:, 0:3, :],
                    )
            nc.sync.dma_start(
                out[b].rearrange("(r p c) -> p r c", r=NCHUNK, p=128)[:, 3:5, :],
                outt[:, 3:5, :],
            )

        # ---- software pipeline over 2-sample groups ----
        # Warmup squares/xT copies ride the then-idle DVE so the Pool engine
        # (DMA gen + steady-state squares/copies) isn't the ramp bottleneck.
        xbgs = {0: xbg0}
        c_sel = const_p.tile([32 + 2 * NCHUNK, NCHUNK * 128], BF16)
        nc.sync.dma_start(c_sel[:], sel_in[:])
        sq2s = {}
        heads = {}
        heads.update(head_pair_split(0, xbgs[0]))
        xbgs[1] = in_dma(1)
        sq2s[1] = emit_sq_pair(1, xbgs[1], eng=nc.vector)
        heads.update(head_pair(1, xbgs[1], sq2s.pop(1), xt_eng=nc.vector))
        xbgs[2] = in_dma(2)
        sq2s[2] = emit_sq_pair(2, xbgs[2])
        xbgs[3] = in_dma(3)
        sq2s[3] = emit_sq_pair(3, xbgs[3])
        sts = {}
        for n in range(B_CORE):
            g = n // 2
            z = zp.tile([128, NCHUNK, DIM], F16, tag="z")
            head = heads.pop(n)
            emit_sample_c(n, head, z)
            sts[n] = {"n": n, "z": z, "rowsum_pair": head[4]}
            if n % 2 == 0:
                if g + 2 < NG:
                    heads.update(head_pair(g + 2, xbgs[g + 2], sq2s.pop(g + 2)))
                if g + 4 < NG:
                    xbgs[g + 4] = in_dma(g + 4)
                    sq2s[g + 4] = emit_sq_pair(g + 4, xbgs[g + 4])
            elif g >= 1:
                tail_pair(g - 1, sts)
        tail_single_stats(B_CORE - 2, sts)
        tail_single_stats(B_CORE - 1, sts, last=True)
        tail_single_z(B_CORE - 2, sts)
        tail_single_z(B_CORE - 1, sts, last=True)

    nc.compile()
    return nc


def _get_nc():
    global _cached_nc
    if _cached_nc is None:
        _cached_nc = build()
    return _cached_nc


def make_in_maps(x: np.ndarray, t: np.ndarray):
    import ml_dtypes

    alpha = float(np.exp(t.astype(np.float64))[0, 0])
    consts = np.zeros((128, 2), dtype=np.float32)
    consts[:, 0] = -2.0 * alpha
    consts[:, 1] = 2.0 * alpha
    ident = np.eye(128, dtype=ml_dtypes.bfloat16)
    sel = np.zeros((32 + 2 * NCHUNK, NCHUNK * 128), dtype=ml_dtypes.bfloat16)
    for base in (0, 32):
        for j in range(NCHUNK):
            sel[base + j, j * 128 : (j + 1) * 128] = 1.0
            sel[base + NCHUNK + j, j * 128 : (j + 1) * 128] = 1.0
    xs = x.reshape(N_CORES, B_CORE, DIM, M)
    return [
        {"x": np.ascontiguousarray(xs[c]), "consts": consts, "ident": ident,
         "sel": sel}
        for c in range(N_CORES)
    ]


def kernel(x: np.ndarray, t: np.ndarray) -> np.ndarray:
    x = np.asarray(x, dtype=np.float32)
    t = np.asarray(t, dtype=np.float32)
    nc = _get_nc()
    res = run_bass_kernel_spmd(nc, make_in_maps(x, t), core_ids=list(range(N_CORES)))
    return np.concatenate(
        [r["out"].astype(np.float32) for r in res.results], axis=0
    )
